# revision 2
# baseline (speedup 1.0000x reference)
"""Trainium2 Bass kernel for nn_Cholesky_from_z — v9.

v7 (parity pair-scan, combined-tile feeds) plus:
  * host packs one DRAM tensor laid out per chunk as [zO_win | zE], so
    each (block, chunk) needs ONE DMA dispatch;
  * dummy 1-col Square/Sqrt at the head of the ACT queue so both
    activation tables load during the initial DMA wait;
  * WE = zE*saE computed in the feed phase (off the post-scan critical
    path); post-scan drains are just ltO = zO*Q and ltE = WE*Q_shift,
    with ltO issued first (it only needs the scan result).

Per [128 x C] pair chunk:
    sa  = Sqrt(1 - Square([zO_win | zE]))        (ACT x2, one op each)
    A   = saE * saO ; WE = zE * saE              (DVE tt x2)
    Q   = segscan(A, bm)                         (DVE scan, half slots)
    ltO = zO * Q ; ltE = WE * Q_shift            (DVE tt x2)
"""

import sys

if "/opt/trn_rl_repo" not in sys.path:
    sys.path.insert(0, "/opt/trn_rl_repo")

import numpy as np

B = 2048
N = 128
NZ = N * (N - 1) // 2          # 8128
NCORES = 8
B_CORE = B // NCORES           # 256
STREAM = 8322                  # padded packed stream (even)
H = STREAM // 2                # 4161 pairs
CHUNKS = [640, 896, 1344, 1281]
CHUNK_OFF = [0, 640, 1536, 2880]
NCH = len(CHUNKS)
assert sum(CHUNKS) == H
# per-chunk combined [zO_win | zE] column offsets in the packed DRAM tensor
ZEO_OFF = []
_o = 0
for _c in CHUNKS:
    ZEO_OFF.append(_o)
    _o += 2 * _c + 1
ZEO_W = _o                     # 8326

# --- host-side stream maps --------------------------------------------------
slot_src = np.full(STREAM, -1, np.int64)
slot_dst = np.full(STREAM, -1, np.int64)
_starts = []
_pos = 1
_zcol = 0
for _i in range(N):
    _starts.append(_pos)
    for _j in range(_i):
        slot_src[_pos] = _zcol + _j
        slot_dst[_pos] = _i * N + _j
        _pos += 1
    slot_dst[_pos] = _i * N + _i
    _pos += 1
    _zcol += _i
    if (_i + 1) % 2 == 1:
        _pos += 1
assert _pos == 8321, _pos

_bm_host = np.zeros(H, np.float16)
_bm_host[(np.asarray(_starts) - 1) // 2] = 1.0
_BM = np.ascontiguousarray(np.broadcast_to(_bm_host, (128, H)))

_src_mask = slot_src >= 0
_dst_mask = slot_dst >= 0

_prog_cache = {}


def _build_program():
    import concourse.bacc as bacc
    import concourse.mybir as mybir
    from concourse.tile import TileContext

    f16 = mybir.dt.float16
    f32 = mybir.dt.float32
    Alu = mybir.AluOpType
    Act = mybir.ActivationFunctionType

    nc = bacc.Bacc("TRN2", target_bir_lowering=False, debug=False,
                   num_devices=NCORES)
    zeo_d = nc.dram_tensor("zeo", [B_CORE, ZEO_W], f16,
                           kind="ExternalInput").ap()
    bm = nc.dram_tensor("bm", [128, H], f16, kind="ExternalInput").ap()
    lpE = nc.dram_tensor("lpE", [B_CORE, H], f16, kind="ExternalOutput").ap()
    lpO = nc.dram_tensor("lpO", [B_CORE, H], f16, kind="ExternalOutput").ap()

    NBLK = B_CORE // 128       # 2
    with TileContext(nc) as tc:
        with (
            tc.tile_pool(name="io", bufs=1) as io_pool,
            tc.tile_pool(name="up", bufs=3) as u_pool,
            tc.tile_pool(name="sp", bufs=2) as s_pool,
            tc.tile_pool(name="apl", bufs=2) as a_pool,
            tc.tile_pool(name="qp", bufs=1) as q_pool,
            tc.tile_pool(name="lt", bufs=2) as lt_pool,
            tc.tile_pool(name="bp", bufs=1) as b_pool,
        ):
            # tiny tile for dummy activations (forces both ACT table loads
            # while the first input DMA is still in flight)
            warm = b_pool.tile([128, 1], f32, tag="warm")
            nc.vector.memset(warm, 1.0)
            nc.scalar.activation(warm, warm, Act.Square)
            nc.scalar.activation(warm, warm, Act.Sqrt, bias=1.0, scale=-1.0)

            zeo = {}
            bmt = None
            for ch, (C, c0) in enumerate(zip(CHUNKS, ZEO_OFF)):
                for blk in range(NBLK):
                    r0 = blk * 128
                    t = io_pool.tile([128, 2 * C + 1], f16, tag=f"z{blk}_{ch}")
                    nc.sync.dma_start(out=t,
                                      in_=zeo_d[r0:r0 + 128, c0:c0 + 2 * C + 1])
                    zeo[blk, ch] = t
                if ch == 1:
                    bmt = b_pool.tile([128, H], f16, tag="bm")
                    nc.sync.dma_start(out=bmt, in_=bm)

            sa, WE = {}, {}
            Qbig = {}
            for blk in range(NBLK):
                qbig_t = q_pool.tile([128, H + 1], f16, tag=f"Q{blk}")
                nc.vector.memset(qbig_t[:, 0:1], 0.0)
                Qbig[blk] = qbig_t

            def feed(blk, ch):
                """ACT: one Square + one fused Sqrt over [zO_win | zE]."""
                C = CHUNKS[ch]
                u = u_pool.tile([128, 2 * C + 1], f32, tag="u")
                nc.scalar.activation(u, zeo[blk, ch], Act.Square)
                s = s_pool.tile([128, 2 * C + 1], f16, tag=f"s{blk}")
                nc.scalar.activation(s, u, Act.Sqrt, bias=1.0, scale=-1.0)
                sa[blk, ch] = s     # [:, 0:C] = saE, [:, C+1:2C+1] = saO

            def scan(blk, ch):
                """DVE: A = saE*saO ; WE = zE*saE ; segscan -> Qbig."""
                C = CHUNKS[ch]
                c0 = CHUNK_OFF[ch]
                s = sa[blk, ch]
                A = a_pool.tile([128, C], f16, tag=f"A{blk}")
                nc.vector.tensor_mul(A, s[:, 0:C], s[:, C + 1:2 * C + 1])
                t = Qbig[blk]
                init = 1.0 if ch == 0 else t[:, c0:c0 + 1]
                nc.vector.tensor_tensor_scan(t[:, c0 + 1:c0 + C + 1], A,
                                             bmt[:, c0:c0 + C], init,
                                             Alu.mult, Alu.add)
                w = a_pool.tile([128, C], f16, tag=f"W{blk}")
                nc.vector.tensor_mul(w, zeo[blk, ch][:, C + 1:2 * C + 1],
                                     s[:, 0:C])
                WE[blk, ch] = w

            def drain(blk, ch):
                """DVE: ltO = zO*Q ; ltE = WE*Q_shift."""
                C = CHUNKS[ch]
                c0 = CHUNK_OFF[ch]
                r0 = blk * 128
                t = Qbig[blk]
                lO = lt_pool.tile([128, C], f16, tag=f"lO{blk}")
                nc.vector.tensor_mul(lO, zeo[blk, ch][:, 1:C + 1],
                                     t[:, c0 + 1:c0 + C + 1])
                nc.sync.dma_start(out=lpO[r0:r0 + 128, c0:c0 + C], in_=lO)
                lE = lt_pool.tile([128, C], f16, tag=f"lE{blk}")
                nc.vector.tensor_mul(lE, WE[blk, ch], t[:, c0:c0 + C])
                nc.sync.dma_start(out=lpE[r0:r0 + 128, c0:c0 + C], in_=lE)

            for ch in range(NCH):
                for blk in range(NBLK):
                    feed(blk, ch)
                for blk in range(NBLK):
                    scan(blk, ch)
                    if ch >= 1:
                        drain(blk, ch - 1)
            for blk in range(NBLK):
                drain(blk, NCH - 1)
    nc.compile()
    return nc


def _get_program():
    if "nc" not in _prog_cache:
        _prog_cache["nc"] = _build_program()
    return _prog_cache["nc"]


def _run(in_maps, **kw):
    from concourse.bass_utils import run_bass_kernel_spmd

    nc = _get_program()
    return run_bass_kernel_spmd(nc, in_maps, list(range(NCORES)), **kw)


def kernel(inputs: np.ndarray, _return_raw=False, **run_kw) -> np.ndarray:
    assert inputs.shape == (B, NZ), inputs.shape
    zvec = np.ascontiguousarray(inputs, dtype=np.float32)

    stream = np.ones((B, STREAM), np.float16)
    stream[:, _src_mask] = zvec[:, slot_src[_src_mask]].astype(np.float16)
    zE = stream[:, 0::2]                                          # (B, H)
    zO = np.concatenate([np.ones((B, 1), np.float16),
                         stream[:, 1::2]], axis=1)                # (B, H+1)
    zeo = np.empty((B, ZEO_W), np.float16)
    for c, (C, c0) in enumerate(zip(CHUNKS, CHUNK_OFF)):
        o = ZEO_OFF[c]
        zeo[:, o:o + C + 1] = zO[:, c0:c0 + C + 1]
        zeo[:, o + C + 1:o + 2 * C + 1] = zE[:, c0:c0 + C]

    in_maps = [
        {"zeo": np.ascontiguousarray(zeo[c * B_CORE:(c + 1) * B_CORE]),
         "bm": _BM}
        for c in range(NCORES)
    ]
    res = _run(in_maps, **run_kw)

    lp_full = np.empty((B, STREAM), np.float32)
    for c in range(NCORES):
        sl = slice(c * B_CORE, (c + 1) * B_CORE)
        lp_full[sl, 0::2] = res.results[c]["lpE"].astype(np.float32)
        lp_full[sl, 1::2] = res.results[c]["lpO"].astype(np.float32)

    out = np.zeros((B, N * N), np.float32)
    out[:, slot_dst[_dst_mask]] = lp_full[:, _dst_mask]
    out = out.reshape(B, N, N)
    if _return_raw:
        return out, res
    return out


# revision 3
# speedup vs baseline: 1.0196x; 1.0196x over previous
"""Trainium2 Bass kernel for nn_Cholesky_from_z — v9.

v7 (parity pair-scan, combined-tile feeds) plus:
  * host packs one DRAM tensor laid out per chunk as [zO_win | zE], so
    each (block, chunk) needs ONE DMA dispatch;
  * dummy 1-col Square/Sqrt at the head of the ACT queue so both
    activation tables load during the initial DMA wait;
  * WE = zE*saE computed in the feed phase (off the post-scan critical
    path); post-scan drains are just ltO = zO*Q and ltE = WE*Q_shift,
    with ltO issued first (it only needs the scan result).

Per [128 x C] pair chunk:
    sa  = Sqrt(1 - Square([zO_win | zE]))        (ACT x2, one op each)
    A   = saE * saO ; WE = zE * saE              (DVE tt x2)
    Q   = segscan(A, bm)                         (DVE scan, half slots)
    ltO = zO * Q ; ltE = WE * Q_shift            (DVE tt x2)
"""

import sys

if "/opt/trn_rl_repo" not in sys.path:
    sys.path.insert(0, "/opt/trn_rl_repo")

import numpy as np

B = 2048
N = 128
NZ = N * (N - 1) // 2          # 8128
NCORES = 8
B_CORE = B // NCORES           # 256
STREAM = 8322                  # padded packed stream (even)
H = STREAM // 2                # 4161 pairs
CHUNKS = [512, 832, 1408, 1409]
CHUNK_OFF = [0, 512, 1344, 2752]
NCH = len(CHUNKS)
assert sum(CHUNKS) == H
# per-chunk combined [zO_win | zE] column offsets in the packed DRAM tensor
ZEO_OFF = []
_o = 0
for _c in CHUNKS:
    ZEO_OFF.append(_o)
    _o += 2 * _c + 1
ZEO_W = _o                     # 8326

# --- host-side stream maps --------------------------------------------------
slot_src = np.full(STREAM, -1, np.int64)
slot_dst = np.full(STREAM, -1, np.int64)
_starts = []
_pos = 1
_zcol = 0
for _i in range(N):
    _starts.append(_pos)
    for _j in range(_i):
        slot_src[_pos] = _zcol + _j
        slot_dst[_pos] = _i * N + _j
        _pos += 1
    slot_dst[_pos] = _i * N + _i
    _pos += 1
    _zcol += _i
    if (_i + 1) % 2 == 1:
        _pos += 1
assert _pos == 8321, _pos

_bm_host = np.zeros(H, np.float16)
_bm_host[(np.asarray(_starts) - 1) // 2] = 1.0
_BM = np.ascontiguousarray(np.broadcast_to(_bm_host, (128, H)))

_src_mask = slot_src >= 0
_dst_mask = slot_dst >= 0

_prog_cache = {}


def _build_program():
    import concourse.bacc as bacc
    import concourse.mybir as mybir
    from concourse.tile import TileContext

    f16 = mybir.dt.float16
    f32 = mybir.dt.float32
    Alu = mybir.AluOpType
    Act = mybir.ActivationFunctionType

    nc = bacc.Bacc("TRN2", target_bir_lowering=False, debug=False,
                   num_devices=NCORES)
    zeo_d = nc.dram_tensor("zeo", [B_CORE, ZEO_W], f16,
                           kind="ExternalInput").ap()
    bm = nc.dram_tensor("bm", [128, H], f16, kind="ExternalInput").ap()
    lpE = nc.dram_tensor("lpE", [B_CORE, H], f16, kind="ExternalOutput").ap()
    lpO = nc.dram_tensor("lpO", [B_CORE, H], f16, kind="ExternalOutput").ap()

    NBLK = B_CORE // 128       # 2
    with TileContext(nc) as tc:
        with (
            tc.tile_pool(name="io", bufs=1) as io_pool,
            tc.tile_pool(name="up", bufs=3) as u_pool,
            tc.tile_pool(name="sp", bufs=2) as s_pool,
            tc.tile_pool(name="apl", bufs=2) as a_pool,
            tc.tile_pool(name="qp", bufs=1) as q_pool,
            tc.tile_pool(name="lt", bufs=2) as lt_pool,
            tc.tile_pool(name="bp", bufs=1) as b_pool,
        ):
            # tiny tile for dummy activations (forces both ACT table loads
            # while the first input DMA is still in flight)
            warm = b_pool.tile([128, 1], f32, tag="warm")
            nc.vector.memset(warm, 1.0)
            nc.scalar.activation(warm, warm, Act.Square)
            nc.scalar.activation(warm, warm, Act.Sqrt, bias=1.0, scale=-1.0)

            zeo = {}
            bmt = None
            for ch, (C, c0) in enumerate(zip(CHUNKS, ZEO_OFF)):
                for blk in range(NBLK):
                    r0 = blk * 128
                    t = io_pool.tile([128, 2 * C + 1], f16, tag=f"z{blk}_{ch}")
                    nc.sync.dma_start(out=t,
                                      in_=zeo_d[r0:r0 + 128, c0:c0 + 2 * C + 1])
                    zeo[blk, ch] = t
                if ch == 1:
                    bmt = b_pool.tile([128, H], f16, tag="bm")
                    nc.sync.dma_start(out=bmt, in_=bm)

            sa, WE = {}, {}
            Qbig = {}
            for blk in range(NBLK):
                qbig_t = q_pool.tile([128, H + 1], f16, tag=f"Q{blk}")
                nc.vector.memset(qbig_t[:, 0:1], 0.0)
                Qbig[blk] = qbig_t

            def feed(blk, ch):
                """ACT: one Square + one fused Sqrt over [zO_win | zE]."""
                C = CHUNKS[ch]
                u = u_pool.tile([128, 2 * C + 1], f32, tag="u")
                nc.scalar.activation(u, zeo[blk, ch], Act.Square)
                s = s_pool.tile([128, 2 * C + 1], f16, tag=f"s{blk}")
                nc.scalar.activation(s, u, Act.Sqrt, bias=1.0, scale=-1.0)
                sa[blk, ch] = s     # [:, 0:C] = saE, [:, C+1:2C+1] = saO

            def scan(blk, ch):
                """DVE: A = saE*saO ; WE = zE*saE ; segscan -> Qbig."""
                C = CHUNKS[ch]
                c0 = CHUNK_OFF[ch]
                s = sa[blk, ch]
                A = a_pool.tile([128, C], f16, tag=f"A{blk}")
                nc.vector.tensor_mul(A, s[:, 0:C], s[:, C + 1:2 * C + 1])
                t = Qbig[blk]
                init = 1.0 if ch == 0 else t[:, c0:c0 + 1]
                nc.vector.tensor_tensor_scan(t[:, c0 + 1:c0 + C + 1], A,
                                             bmt[:, c0:c0 + C], init,
                                             Alu.mult, Alu.add)
                w = a_pool.tile([128, C], f16, tag=f"W{blk}")
                nc.vector.tensor_mul(w, zeo[blk, ch][:, C + 1:2 * C + 1],
                                     s[:, 0:C])
                WE[blk, ch] = w

            def drain(blk, ch):
                """DVE: ltO = zO*Q ; ltE = WE*Q_shift."""
                C = CHUNKS[ch]
                c0 = CHUNK_OFF[ch]
                r0 = blk * 128
                t = Qbig[blk]
                lO = lt_pool.tile([128, C], f16, tag=f"lO{blk}")
                nc.vector.tensor_mul(lO, zeo[blk, ch][:, 1:C + 1],
                                     t[:, c0 + 1:c0 + C + 1])
                nc.sync.dma_start(out=lpO[r0:r0 + 128, c0:c0 + C], in_=lO)
                lE = lt_pool.tile([128, C], f16, tag=f"lE{blk}")
                nc.vector.tensor_mul(lE, WE[blk, ch], t[:, c0:c0 + C])
                nc.sync.dma_start(out=lpE[r0:r0 + 128, c0:c0 + C], in_=lE)

            for ch in range(NCH):
                for blk in range(NBLK):
                    feed(blk, ch)
                if ch >= 1:
                    for blk in range(NBLK):
                        drain(blk, ch - 1)
                for blk in range(NBLK):
                    scan(blk, ch)
            for blk in range(NBLK):
                drain(blk, NCH - 1)
    nc.compile()
    return nc


def _get_program():
    if "nc" not in _prog_cache:
        _prog_cache["nc"] = _build_program()
    return _prog_cache["nc"]


def _run(in_maps, **kw):
    from concourse.bass_utils import run_bass_kernel_spmd

    nc = _get_program()
    return run_bass_kernel_spmd(nc, in_maps, list(range(NCORES)), **kw)


def kernel(inputs: np.ndarray, _return_raw=False, **run_kw) -> np.ndarray:
    assert inputs.shape == (B, NZ), inputs.shape
    zvec = np.ascontiguousarray(inputs, dtype=np.float32)

    stream = np.ones((B, STREAM), np.float16)
    stream[:, _src_mask] = zvec[:, slot_src[_src_mask]].astype(np.float16)
    zE = stream[:, 0::2]                                          # (B, H)
    zO = np.concatenate([np.ones((B, 1), np.float16),
                         stream[:, 1::2]], axis=1)                # (B, H+1)
    zeo = np.empty((B, ZEO_W), np.float16)
    for c, (C, c0) in enumerate(zip(CHUNKS, CHUNK_OFF)):
        o = ZEO_OFF[c]
        zeo[:, o:o + C + 1] = zO[:, c0:c0 + C + 1]
        zeo[:, o + C + 1:o + 2 * C + 1] = zE[:, c0:c0 + C]

    in_maps = [
        {"zeo": np.ascontiguousarray(zeo[c * B_CORE:(c + 1) * B_CORE]),
         "bm": _BM}
        for c in range(NCORES)
    ]
    res = _run(in_maps, **run_kw)

    lp_full = np.empty((B, STREAM), np.float32)
    for c in range(NCORES):
        sl = slice(c * B_CORE, (c + 1) * B_CORE)
        lp_full[sl, 0::2] = res.results[c]["lpE"].astype(np.float32)
        lp_full[sl, 1::2] = res.results[c]["lpO"].astype(np.float32)

    out = np.zeros((B, N * N), np.float32)
    out[:, slot_dst[_dst_mask]] = lp_full[:, _dst_mask]
    out = out.reshape(B, N, N)
    if _return_raw:
        return out, res
    return out
